# revision 25
# baseline (speedup 1.0000x reference)
"""KMeans assignment kernel for TRN2 (8 NeuronCores, data-parallel over points).

Computes argmin_k ||x_n - c_k||^2 for x (65536, 512) f32, centers (4096, 512) f32.

Strategy (single-pass fp32r, ~3x less PE work than the 3-pass hi/lo scheme):
  - argmin_k dist = argmax_k s,  s = 2*x.c_k - ||c_k||^2   (x-norm constant per row)
  - ONE matmul pass p = (2x) @ c^T in fp32r (e8m11, full PE rate). The e8m11
    rounding perturbs each score by sigma ~ 9e-3 while the top1-top2 gap is
    > 0.13 for 99% of points: 18/65536 argmax flips on the actual data
    (rel err 1.1e-2, under the 2e-2 gate).
  - The -||c_k||^2 bias enters as the START matmul of each bank's PSUM
    accumulation group: ones[2,128] stationary x [bias_hi; bias_lo] moving
    (hi/lo fp32r split keeps the bias exact to ~6e-5). A proper start=True
    group is the only PSUM-init the tile scheduler orders correctly —
    engine-side preloads race the PE (verified: nondeterministic cold-start
    corruption; manual semaphore repairs can deadlock the device).
  - Per half (4 banks): bias matmuls back-to-back (ones stationary loads
    once), then fc-outer so each x chunk stays stationary across 4 banks.
  - Act evacuates each PSUM half to a s[128,2,4,512] SBUF tile (releases
    PSUM early; no PE stall on WAR).
  - Argmax: DVE tensor_reduce -> 8 block maxes, top8 sorts them (global max
    at slot 0), then ONE max_index scan over s finds the first occurrence =
    exact argmin index with jnp-compatible tie-breaking.
  - Data-parallel: 8192 points/core, centers replicated; no collectives.
"""
import os
import numpy as np

import concourse.bass as bass
import concourse.bacc as bacc
import concourse.tile as tile
import concourse.mybir as mybir
from concourse.bass_utils import run_bass_kernel_spmd

N_CORES = 8
N_POINTS = 65536
K = 4096
F = 512
PTS_PER_CORE = N_POINTS // N_CORES      # 8192
NT = PTS_PER_CORE // 128                # 64 x-tiles per core
NFC = F // 128                          # 4 contraction chunks
NQ = 4                                  # PSUM quarters
KQ = K // NQ                            # 1024 centers per quarter
F32 = mybir.dt.float32
F32R = mybir.dt.float32r
U32 = mybir.dt.uint32
ALU = mybir.AluOpType

_NC = None
LAST_BR = None


def round_fp32r(a: np.ndarray) -> np.ndarray:
    """Round f32 to fp32r (e8m11): RNE to 11 mantissa bits; low 12 bits zero."""
    bits = np.ascontiguousarray(a, dtype=np.float32).view(np.uint32)
    rounded = (bits.astype(np.uint64) + 0x7FF + ((bits >> 12) & 1)) & 0xFFFFF000
    return rounded.astype(np.uint32).view(np.float32)


def _build():
    nc = bacc.Bacc("TRN2", target_bir_lowering=False, debug=False,
                   num_devices=N_CORES)
    xh_d = nc.declare_dram_parameter("xh", [NT, 128, NFC, 128], F32R, isOutput=False)
    ch_d = nc.declare_dram_parameter("ch", [NFC, 128, K], F32R, isOutput=False)
    cnn_d = nc.declare_dram_parameter("cnn", [2, K], F32R, isOutput=False)
    one_d = nc.declare_dram_parameter("one2", [2, 128], F32R, isOutput=False)
    out_d = nc.declare_dram_parameter("oidx", [128, NT], U32, isOutput=True)

    NB = 4                              # banks per PSUM half
    with tile.TileContext(nc) as tc:
        with (
            tc.tile_pool(name="const", bufs=1) as cpool,
            tc.tile_pool(name="xp", bufs=4) as xpool,
            tc.tile_pool(name="sp", bufs=3) as spool,
            tc.tile_pool(name="m8p", bufs=2) as m8pool,
            tc.tile_pool(name="st", bufs=1) as stpool,
            tc.tile_pool(name="ps", bufs=1, space="PSUM") as pspool,
        ):
            # [bias_hi; bias_lo] fp32r rows of -||c||^2; the hi/lo pair keeps
            # the bias exact to ~6e-5 despite the 11-bit fp32r mantissa.
            cnn = cpool.tile([2, K], F32R, tag="cnn")
            nc.sync.dma_start(cnn[:], cnn_d[:])
            ones2 = cpool.tile([2, 128], F32R, tag="ones2")
            nc.sync.dma_start(ones2[:], one_d[:])
            # Spread the 4x4MB center loads across four DMA trigger queues so
            # they land in ~1/4 the serial time (the first tile's matmuls wait
            # on all of them).
            chs = []
            ch_engines = [nc.scalar, nc.gpsimd, nc.scalar, nc.gpsimd]
            for fc in range(NFC):
                cht = cpool.tile([128, K], F32R, tag=f"ch{fc}", name=f"ch{fc}")
                ch_engines[fc].dma_start(cht[:], ch_d[fc])
                chs.append(cht)

            ist = stpool.tile([128, NT, 8], U32, tag="ist")

            for t in range(NT):
                xt = xpool.tile([128, NFC * 128], F32R, tag="x")
                nc.sync.dma_start(xt[:], xh_d[t])

                s = spool.tile([128, 2, NB, 512], F32, tag="s")
                for h in range(2):
                    ph = pspool.tile([128, NB, 512], F32, tag=f"p{h}",
                                     name=f"p{h}")
                    def ks(b):
                        return slice(h * 2048 + b * 512, h * 2048 + (b + 1) * 512)
                    # Bias matmuls open each bank's accumulation group: a
                    # proper start=True group is the only PSUM-init the tile
                    # scheduler tracks (engine preloads race the PE). Grouped
                    # back-to-back so the ones-stationary is loaded once.
                    for b in range(NB):
                        nc.tensor.matmul(
                            ph[:, b, :], ones2[:], cnn[:, ks(b)],
                            start=True, stop=False)
                    # fc-outer: each xt chunk stays stationary across 4 banks.
                    for fc in range(NFC):
                        for b in range(NB):
                            nc.tensor.matmul(
                                ph[:, b, :],
                                xt[:, fc * 128:(fc + 1) * 128],
                                chs[fc][:, ks(b)],
                                start=False,
                                stop=(fc == NFC - 1),
                            )
                    # evacuate PSUM half to SBUF (frees it for tile t+1)
                    nc.scalar.copy(s[:, h], ph[:])

                # DVE: block maxes -> sorted top8 -> ONE exact argmax scan
                m8 = m8pool.tile([128, 8], F32, tag="m8")
                m8s = m8pool.tile([128, 8], F32, tag="m8s")
                nc.vector.tensor_reduce(
                    out=m8[:], in_=s.rearrange("p h b f -> p (h b) f"),
                    axis=mybir.AxisListType.X, op=ALU.max)
                nc.vector.max(m8s[:], m8[:])
                nc.vector.max_index(ist[:, t, :], m8s[:],
                                    s.rearrange("p h b f -> p (h b f)"))

            ex = stpool.tile([128, NT], U32, tag="ex")
            nc.vector.tensor_copy(out=ex[:], in_=ist[:, :, 0])
            nc.gpsimd.dma_start(out_d[:], ex[:])
    nc.compile()
    return nc


def _get_nc():
    global _NC
    if _NC is None:
        _NC = _build()
    return _NC


def kernel(x: np.ndarray, centers: np.ndarray) -> np.ndarray:
    global LAST_BR, _LAST_IN_MAPS
    x = np.ascontiguousarray(x, dtype=np.float32)
    centers = np.ascontiguousarray(centers, dtype=np.float32)

    v_hi = round_fp32r((2.0 * x).astype(np.float32))
    c_hi = round_fp32r(centers)

    # pack x side: [core, t, fp, fc, j] <- v[core*8192 + t*128 + j, fc*128 + fp]
    a = v_hi.reshape(N_CORES, NT, 128, NFC, 128)      # [core, t, j, fc, fp]
    xh_p = np.ascontiguousarray(a.transpose(0, 1, 4, 3, 2))

    # pack c side: [fc, fp, k] <- c[k, fc*128 + fp]
    c = c_hi.reshape(K, NFC, 128)                     # [k, fc, fp]
    ch_p = np.ascontiguousarray(c.transpose(1, 2, 0))

    bias = (-(centers.astype(np.float64) ** 2).sum(axis=1)).astype(np.float32)
    b_hi = round_fp32r(bias)
    b_lo = round_fp32r((bias - b_hi).astype(np.float32))
    cnn_p = np.ascontiguousarray(np.stack([b_hi, b_lo], axis=0))  # (2, K)

    one2 = np.ones((2, 128), dtype=np.float32)
    in_maps = [
        {"xh": xh_p[i], "ch": ch_p, "cnn": cnn_p, "one2": one2}
        for i in range(N_CORES)
    ]

    nc = _get_nc()
    _LAST_IN_MAPS = in_maps
    br = run_bass_kernel_spmd(nc, in_maps, list(range(N_CORES)))
    LAST_BR = br

    parts = []
    for i in range(N_CORES):
        oidx = br.results[i]["oidx"]                  # (128, NT) u32
        parts.append(oidx.T.reshape(-1))              # point-major
    return np.concatenate(parts).astype(np.int32)


_LAST_IN_MAPS = None


def _install_ntff_shim():
    """antenv.axon_hooks is missing in some images; rebuild it from the boot
    helper so run_bass_kernel_spmd(trace=True) can profile via NTFF."""
    import sys, types
    try:
        from antenv.axon_hooks import get_axon_ntff_profile_hook  # noqa: F401
        return True
    except ImportError:
        pass
    try:
        from trn_agent_boot.trn_boot import _ntff_profile_via_ctypes
        hook = _ntff_profile_via_ctypes('/opt/axon/libaxon_pjrt.so')
        mod = types.ModuleType("antenv.axon_hooks")
        mod.get_axon_ntff_profile_hook = lambda: hook
        mod.set_axon_ntff_profile_hook = lambda h: None
        sys.modules["antenv.axon_hooks"] = mod
        return True
    except Exception:
        return False


def measure_exec_ns(reps: int = 1) -> int:
    """Real HW execution time from a neuron-profile (NTFF) capture."""
    import tempfile
    nc = _get_nc()
    assert _LAST_IN_MAPS is not None, "call kernel() first"
    _install_ntff_shim()
    tmpdir = tempfile.mkdtemp(prefix="kmeans_ntff_")
    br = run_bass_kernel_spmd(nc, _LAST_IN_MAPS, list(range(N_CORES)),
                              trace=True, tmpdir=tmpdir)
    assert br.exec_time_ns is not None, "NTFF profiling produced no timing"
    return int(br.exec_time_ns)
